# revision 8
# baseline (speedup 1.0000x reference)
"""Trainium2 Bass kernel for nn_CAM (DANet channel-attention module).

Per batch element b (one per NeuronCore, 8 cores data-parallel over B=8):
    xf = x[b].reshape(C, H*W)                       # [512, 4096]
    E = xf @ xf.T                                   # [512, 512] (symmetric)
    att = softmax(max_j(E) - E, axis=-1)            # inverted softmax
    out = gamma * (att @ xf) + x[b]

Kernel math (identical in exact arithmetic to the reference):
    c[i]    = min_j E[i, j]         (= column min by symmetry)
    W[j, i] = exp(c[i] - E[j, i])   (= numerator of att[i, j]; exponent <= 0)
    S[i]    = sum_j W[j, i]
    out[i]  = gamma * (1/S[i]) * sum_j W[j, i] * xf[j, :] + x[b][i, :]

Layout strategy:
  - xf natural  [c_part, n_free]  : [128, 4, 4096] f32 (residual + source)
  - xf^T        [n_part, c_free]  : [128, 32, 512] f32r via PE transposes
  - E           [j_part, i_free]  : 4 PSUM banks, fp32r matmuls over 32 k-tiles
  - W           [j_part, i_free]  : f32r; serves directly as lhsT of matmul2
                                    (no attention transpose needed, E symmetry)
  - Xr          f32r copy of xf for matmul2 rhs; shares XT's SBUF slot
  - S via ones-matmuls landing [i_part, 8] -> per-partition reciprocal

reps > 1 unrolls the whole computation serially inside one NEFF (used by
test.py to measure steady-state per-iteration device time).
"""

import numpy as np

import concourse.bass as bass
import concourse.mybir as mybir
import concourse.tile as tile
from concourse import bacc
from concourse.masks import make_identity

P = 128          # partitions
C = 512          # channels
HW = 4096        # spatial (64*64)
CB = C // P      # 4 channel blocks
KB = HW // P     # 32 spatial blocks
NW = 512         # matmul free-dim chunk
NCH = HW // NW   # 8 n-chunks

F32 = mybir.dt.float32
F32R = mybir.dt.float32r
EXP = mybir.ActivationFunctionType.Exp
ALU = mybir.AluOpType
AX = mybir.AxisListType


def build_nc(reps: int = 1):
    nc = bacc.Bacc("TRN2", target_bir_lowering=False)
    x = nc.dram_tensor("x", [C, HW], F32, kind="ExternalInput")
    g = nc.dram_tensor("gamma", [1], F32, kind="ExternalInput")
    y = nc.dram_tensor("y", [C, HW], F32, kind="ExternalOutput")

    with tile.TileContext(nc) as tc:
        with (
            tc.tile_pool(name="xin", bufs=1) as xin_pool,
            tc.tile_pool(name="xt", bufs=1) as xt_pool,
            tc.tile_pool(name="w", bufs=1) as w_pool,
            tc.tile_pool(name="small", bufs=1) as small,
            tc.tile_pool(name="outp", bufs=2) as outp,
            tc.tile_pool(name="dram", bufs=1, space="DRAM") as dramp,
            tc.tile_pool(name="pxt", bufs=2, space="PSUM") as pxt_pool,
            tc.tile_pool(name="pe", bufs=4, space="PSUM") as pe_pool,
            tc.tile_pool(name="po", bufs=2, space="PSUM") as po_pool,
        ):
            # constants (hoisted out of the rep loop)
            ident = small.tile([P, P], F32)
            make_identity(nc, ident)
            ones_f = small.tile([P, 8], F32)
            nc.vector.memset(ones_f, 1.0)
            ones = small.tile([P, 8], F32R)
            nc.scalar.copy(out=ones, in_=ones_f)
            gamma_bc = small.tile([P, 1], F32)
            nc.gpsimd.dma_start(out=gamma_bc, in_=g[:].partition_broadcast(P))

            xr = x.rearrange("(t p) n -> p t n", p=P)
            yr = y.rearrange("(t p) n -> p t n", p=P)

            for _rep in range(reps):
                X = xin_pool.tile([P, CB, HW], F32, tag="x")
                XT = xt_pool.tile([P, KB, C], F32R, tag="xt")
                W = w_pool.tile([P, CB, C], F32R, tag="w")
                Wtmp = w_pool.tile([P, CB, C], F32, tag="wtmp")
                rowmin = small.tile([P, CB], F32, tag="rowmin")
                c_bc = small.tile([P, NW], F32, tag="cbc")
                invsg = small.tile([P, CB], F32, tag="invsg")
                c_d = dramp.tile([C], F32, tag="cd")

                # ---- load x + build xf^T via PE transposes, n-chunk at a time
                for ch in range(NCH):
                    nsl = slice(ch * NW, (ch + 1) * NW)
                    nc.sync.dma_start(out=X[:, :, nsl], in_=xr[:, :, nsl])
                    for kk in range(NW // P):
                        k = ch * (NW // P) + kk
                        pxt = pxt_pool.tile([P, C], F32, tag="pxt")
                        for t in range(CB):
                            nc.tensor.transpose(
                                pxt[:, t * P:(t + 1) * P],
                                X[:, t, k * P:(k + 1) * P],
                                ident,
                            )
                        nc.scalar.copy(out=XT[:, k, :], in_=pxt)

                # ---- matmul1: E[jb] = sum_k XT_k[:, jb]^T @ XT_k  (fp32r)
                pe_tiles = []
                for jb in range(CB):
                    pe_t = pe_pool.tile([P, C], F32, tag="e")
                    pe_tiles.append(pe_t)
                    for k in range(KB):
                        nc.tensor.matmul(
                            pe_t,
                            lhsT=XT[:, k, jb * P:(jb + 1) * P],
                            rhs=XT[:, k, :],
                            start=(k == 0),
                            stop=(k == KB - 1),
                        )
                    nc.vector.tensor_reduce(
                        out=rowmin[:, jb:jb + 1], in_=pe_t, axis=AX.X, op=ALU.min,
                    )

                # ---- c to free-axis layout via DRAM roundtrip + bcast DMA
                nc.sync.dma_start(
                    out=c_d.rearrange("(t p) -> p t", p=P), in_=rowmin[:, :]
                )
                nc.gpsimd.dma_start(out=c_bc, in_=c_d.partition_broadcast(P))

                # ---- W = exp(c - E)
                for jb in range(CB):
                    nc.vector.tensor_tensor(
                        out=Wtmp[:, jb, :], in0=c_bc, in1=pe_tiles[jb],
                        op=ALU.subtract,
                    )
                    nc.scalar.activation(
                        out=W[:, jb, :], in_=Wtmp[:, jb, :], func=EXP
                    )

                # ---- phase 2: out = gamma * (1/S) * (W^T @ xf) + x
                # Xr (f32r rounded copy for matmul2 rhs) reuses XT's SBUF slot
                Xr = xt_pool.tile([P, CB, HW], F32R, tag="xt")
                for jb in range(CB):
                    for chn in range(NCH):
                        nsl = slice(chn * NW, (chn + 1) * NW)
                        nc.scalar.copy(out=Xr[:, jb, nsl], in_=X[:, jb, nsl])

                for ib in range(CB):
                    isl = slice(ib * P, (ib + 1) * P)
                    ps_t = po_pool.tile([P, 8], F32, tag="o")
                    for jb in range(CB):
                        nc.tensor.matmul(
                            ps_t,
                            lhsT=W[:, jb, isl],
                            rhs=ones,
                            start=(jb == 0),
                            stop=(jb == CB - 1),
                        )
                    nc.vector.reciprocal(out=invsg[:, ib:ib + 1], in_=ps_t[:, 0:1])
                    nc.vector.tensor_tensor(
                        out=invsg[:, ib:ib + 1], in0=invsg[:, ib:ib + 1],
                        in1=gamma_bc, op=ALU.mult,
                    )
                    out_sb = outp.tile([P, HW], F32, tag="osb")
                    for chn in range(NCH):
                        nsl = slice(chn * NW, (chn + 1) * NW)
                        po_t = po_pool.tile([P, NW], F32, tag="o")
                        for jb in range(CB):
                            nc.tensor.matmul(
                                po_t,
                                lhsT=W[:, jb, isl],
                                rhs=Xr[:, jb, nsl],
                                start=(jb == 0),
                                stop=(jb == CB - 1),
                            )
                        nc.vector.scalar_tensor_tensor(
                            out=out_sb[:, nsl],
                            in0=po_t,
                            scalar=invsg[:, ib:ib + 1],
                            in1=X[:, ib, nsl],
                            op0=ALU.mult,
                            op1=ALU.add,
                        )
                    nc.sync.dma_start(out=yr[:, ib, :], in_=out_sb)

    nc.compile()
    return nc


_NC_CACHE = None


def _get_nc():
    global _NC_CACHE
    if _NC_CACHE is None:
        _NC_CACHE = build_nc()
    return _NC_CACHE


def kernel(x, gamma):
    from concourse.bass_utils import run_bass_kernel_spmd

    x = np.ascontiguousarray(np.asarray(x, dtype=np.float32))
    B = x.shape[0]
    assert x.shape == (8, C, 64, 64), x.shape
    xf = x.reshape(B, C, HW)
    gamma = np.ascontiguousarray(np.asarray(gamma, dtype=np.float32)).reshape(1)

    nc = _get_nc()
    in_maps = [{"x": xf[b], "gamma": gamma} for b in range(B)]
    res = run_bass_kernel_spmd(nc, in_maps, core_ids=list(range(B)))
    out = np.stack([res.results[b]["y"] for b in range(B)], axis=0)
    return out.reshape(B, C, 64, 64).astype(np.float32)
